# revision 25
# baseline (speedup 1.0000x reference)
"""MultiHeadSelfAttention TRN2 kernel — head-tensor-parallel over 8 NeuronCores.

Reference semantics (note the quirk: softmax over the QUERY axis):
    Q = x @ Wq[h].T + bq[h]            [B,S,D] per head
    K = x @ Wk[h].T + bk[h]
    V = x @ Wv[h].T + bv[h]
    scores[s,t] = (Q[s]·K[t]) / sqrt(D)
    attn = softmax over s (query axis)  -> attn[s,t] = exp(sc[s,t]) / sum_s' exp(sc[s',t])
    Z[s] = sum_t attn[s,t] V[t]
    out = concat_heads(Z) @ Wo.T + bo

Sharding: head h -> core h. Each core computes its head's partial output
projection out_h = Z_h @ Wo[:, h*D:(h+1)*D].T ; host sums the 8 partials.

bq is dropped entirely: its score contribution bq·K[t] is constant along
the softmax (query) axis and cancels exactly; bk survives via K+bk.

Layout strategy (everything transposed so the quirky softmax normalization
axis 's' lands on the free dimension):
    xT   [d, s]   QT = WqT.T @ xT   [e, s]
    KT   [e, t],  V [t, e]
    scoresT[t, s] = KT.T @ QT  -> exp with ACT accum_out => denom[t] for free
    V'[t,:] = V[t,:] / denom[t]
    ZT[e, s] = V'.T @ PT   (contraction over t, accumulated per t-superblock)
    outT[o, s] = WoHT.T @ ZT
All matmuls run in float32r: full PE rate at 512-wide, and keeping ONE
matmul dtype matters — the PE charges ~180ns extra on every bf16->f32r
fp32_mode switch and the Tile scheduler interleaves phases freely.

Pipeline: per superblock g, scores/exp of g overlap the ZT quarters of
g-1; the next batch's x-load + QKV projections are emitted before the
final superblock's ZT so they fill the exp-tail window; the final
superblock runs sq-major with out-projection chunks lagging one sq so
the zt-copy latency hides under the next quarter's matmuls.

PSUM budget: acc [128,1024] x3 bufs = 6 banks (projections + scores),
z0/z1 [128,512] = 2 banks (ZT superblock accumulation + out-proj).
"""

import numpy as np
import ml_dtypes

import concourse.bass as bass
import concourse.mybir as mybir
import concourse.tile as tile
from concourse import bacc
from concourse.bass_utils import run_bass_kernel_spmd

B, S, D, H = 4, 2048, 256, 8
N_CORES = 8
P = 128          # partitions
NDB = D // P     # 2 d-blocks (contraction blocks for projections)
NTB = S // P     # 16 key/t blocks
SC = 512         # matmul moving-dim chunk == psum tile width
NSC = S // SC    # 4 s chunks
SH = 1024        # s-half (scores psum tile width)
NSH = S // SH    # 2 s halves
G = 4            # t-blocks per superblock (ZT PSUM accumulation group)
NSUP = NTB // G  # 4 superblocks
VG = 4           # V t-blocks per psum alloc

f32 = mybir.dt.float32
f32r = mybir.dt.float32r
bf16 = mybir.dt.bfloat16
EXP = mybir.ActivationFunctionType.Exp


def _build():
    nc = bacc.Bacc(target_bir_lowering=False)

    xT = nc.dram_tensor("xT", [B, D, S], bf16, kind="ExternalInput")
    wqT = nc.dram_tensor("wqT", [D, D], bf16, kind="ExternalInput")  # [d,e] = (Wq/sqrt(D)).T
    wkT = nc.dram_tensor("wkT", [D, D], bf16, kind="ExternalInput")  # [d,e]
    wvT = nc.dram_tensor("wvT", [D, D], bf16, kind="ExternalInput")  # [d,e]
    woT = nc.dram_tensor("woT", [D, D], bf16, kind="ExternalInput")  # [e,o]
    bkc = nc.dram_tensor("bkc", [D, 1], f32, kind="ExternalInput")
    bvb = nc.dram_tensor("bvb", [P, VG * D], f32, kind="ExternalInput")  # bv tiled
    boc = nc.dram_tensor("boc", [D, 1], f32, kind="ExternalInput")  # bo (core0) / zeros
    outT = nc.dram_tensor("outT", [B, D, S], f32, kind="ExternalOutput")

    with tile.TileContext(nc) as tc:
        with (
            tc.tile_pool(name="const", bufs=1) as cpool,
            tc.tile_pool(name="big", bufs=2) as xpool,
            tc.tile_pool(name="qk", bufs=1) as qpool,
            tc.tile_pool(name="zz", bufs=1) as zpool,
            tc.tile_pool(name="pt", bufs=2) as ppool,
            tc.tile_pool(name="small", bufs=2) as spool,
            tc.tile_pool(name="outp", bufs=2) as opool,
            tc.tile_pool(name="ps_a", bufs=3, space="PSUM") as psa,
            tc.tile_pool(name="ps_z", bufs=1, space="PSUM") as psz,
        ):
            # ---- constants (once) ----
            wq_t = cpool.tile([P, NDB, D], bf16, tag="wq")
            wk_t = cpool.tile([P, NDB, D], bf16, tag="wk")
            wv_t = cpool.tile([P, NDB, D], bf16, tag="wv")
            wo_t = cpool.tile([P, NDB, D], bf16, tag="wo")
            nc.scalar.dma_start(
                out=wq_t[:], in_=wqT.rearrange("(n p) e -> p n e", p=P)
            )
            bk_t = cpool.tile([P, NDB, 1], f32, tag="bk")
            bo_t = cpool.tile([P, NDB, 1], f32, tag="bo")
            bvb_t = cpool.tile([P, VG * D], f32, tag="bvb")
            for w_t, w_d in ((wk_t, wkT), (wv_t, wvT)):
                nc.gpsimd.dma_start(
                    out=w_t[:], in_=w_d.rearrange("(n p) e -> p n e", p=P)
                )
            nc.gpsimd.dma_start(
                out=bk_t[:], in_=bkc.rearrange("(n p) o -> p n o", p=P)
            )
            nc.gpsimd.dma_start(out=bvb_t[:], in_=bvb[:])
            nc.gpsimd.dma_start(
                out=wo_t[:], in_=woT.rearrange("(n p) e -> p n e", p=P)
            )
            nc.gpsimd.dma_start(
                out=bo_t[:], in_=boc.rearrange("(n p) o -> p n o", p=P)
            )

            def load_x(b):
                xt = xpool.tile([P, NDB, S], bf16, tag="xt")
                xT_r = xT[b].rearrange("(n p) s -> p n s", p=P)
                if b == 0:
                    # spread the first batch across four queues so the first
                    # projection matmuls start as soon as possible (the scalar
                    # queue is blocked ~1.3us by the ACT table load first)
                    nc.sync.dma_start(out=xt[:, :, 0:SC], in_=xT_r[:, :, 0:SC])
                    nc.scalar.dma_start(out=xt[:, :, SC:SH], in_=xT_r[:, :, SC:SH])
                    nc.sync.dma_start(out=xt[:, :, SH:S], in_=xT_r[:, :, SH:S])
                else:
                    for sh in range(NSH):
                        nc.sync.dma_start(
                            out=xt[:, :, bass.ts(sh, SH)],
                            in_=xT_r[:, :, bass.ts(sh, SH)],
                        )
                return xt

            def proj_qkv(b, xt):
                qt = qpool.tile([P, NDB, S], bf16, tag="qt")
                kt = qpool.tile([P, NDB, S], bf16, tag="kt")
                if b == 0:
                    # sh-major: consume the s-half that arrives first
                    order = [
                        (dst, w, bias, eb, sh)
                        for sh in range(NSH)
                        for dst, w, bias in ((qt, wq_t, None), (kt, wk_t, bk_t))
                        for eb in range(NDB)
                    ]
                else:
                    order = [
                        (dst, w, bias, eb, sh)
                        for dst, w, bias in ((qt, wq_t, None), (kt, wk_t, bk_t))
                        for eb in range(NDB)
                        for sh in range(NSH)
                    ]
                for dst, w, bias, eb, sh in order:
                    ps = psa.tile([P, SH], f32, tag="acc")
                    for sc in range(SH // SC):
                        ssl = bass.ds(sh * SH + sc * SC, SC)
                        psl = bass.ts(sc, SC)
                        for db in range(NDB):
                            nc.tensor.matmul(
                                ps[:, psl],
                                w[:, db, bass.ts(eb, P)],
                                xt[:, db, ssl],
                                start=(db == 0),
                                stop=(db == NDB - 1),
                            )
                        if b == 0:
                            # evacuate per 512-chunk: the first chunks only
                            # depend on the first x pieces
                            if bias is None:
                                nc.vector.tensor_copy(
                                    dst[:, eb, ssl], ps[:, psl]
                                )
                            else:
                                nc.vector.tensor_scalar_add(
                                    dst[:, eb, ssl], ps[:, psl], bias[:, eb, :]
                                )
                    if b != 0:
                        if bias is None:
                            nc.vector.tensor_copy(dst[:, eb, bass.ts(sh, SH)], ps[:])
                        else:
                            nc.vector.tensor_scalar_add(
                                dst[:, eb, bass.ts(sh, SH)], ps[:], bias[:, eb, :]
                            )
                v_all = qpool.tile([P, NTB, D], bf16, tag="v")
                for vg in range(NTB // VG):
                    psv = psa.tile([P, VG * D], f32, tag="acc")
                    for k in range(VG):
                        tb = vg * VG + k
                        for db in range(NDB):
                            nc.tensor.matmul(
                                psv[:, bass.ts(k, D)],
                                xt[:, db, bass.ts(tb, P)],
                                wv_t[:, db, :],
                                start=(db == 0),
                                stop=(db == NDB - 1),
                            )
                    nc.vector.tensor_add(
                        v_all[:, bass.ds(vg * VG, VG), :],
                        psv[:].rearrange("p (g e) -> p g e", g=VG),
                        bvb_t[:].rearrange("p (g e) -> p g e", g=VG),
                    )
                return qt, kt, v_all

            def run_batch(b, qt, kt, v_all, next_cb):
                """scores+exp+norm superblocks interleaved with ZT quarters;
                next_cb() emits the next batch's x-load + projections just
                before the final superblock's ZT so they fill the exp tail."""
                zt = zpool.tile([P, NDB, S], bf16, tag="zt")

                def emit_scores_j(g, tiles, j):
                    pt, vp, dnp = tiles
                    tb = g * G + j
                    for sh in range(NSH):
                        pssc = psa.tile([P, SH], f32, tag="acc")
                        for sc in range(SH // SC):
                            ssl = bass.ds(sh * SH + sc * SC, SC)
                            psl = bass.ts(sc, SC)
                            for eb in range(NDB):
                                nc.tensor.matmul(
                                    pssc[:, psl],
                                    kt[:, eb, bass.ts(tb, P)],
                                    qt[:, eb, ssl],
                                    start=(eb == 0),
                                    stop=(eb == NDB - 1),
                                )
                        nc.scalar.activation(
                            pt[:, j, bass.ts(sh, SH)],
                            pssc[:],
                            EXP,
                            accum_out=dnp[:, j, sh : sh + 1],
                        )

                def emit_norm(g, tiles):
                    _, vp, dnp = tiles
                    dn = spool.tile([P, G, 1], f32, tag="dn")
                    rc = spool.tile([P, G, 1], f32, tag="rc")
                    for j0 in range(0, G, 2):
                        nc.vector.tensor_add(
                            dn[:, j0 : j0 + 2, :],
                            dnp[:, j0 : j0 + 2, 0:1],
                            dnp[:, j0 : j0 + 2, 1:2],
                        )
                        nc.vector.reciprocal(
                            rc[:, j0 : j0 + 2, :], dn[:, j0 : j0 + 2, :]
                        )
                        for jj in (j0, j0 + 1):
                            nc.vector.tensor_scalar_mul(
                                vp[:, jj, :], v_all[:, g * G + jj, :], rc[:, jj, :]
                            )

                def emit_zt_q(g, tiles, qi):
                    pt, vp, _ = tiles
                    eh, sq = qi // NSC, qi % NSC
                    psz_t = psz.tile([P, SC], f32, tag=f"z{sq % 2}")
                    ssl = bass.ts(sq, SC)
                    for j in range(G):
                        nc.tensor.matmul(
                            psz_t[:],
                            vp[:, j, bass.ts(eh, P)],
                            pt[:, j, ssl],
                            start=(j == 0),
                            stop=(j == G - 1),
                        )
                    zsl = zt[:, eh, ssl]
                    if g == 0:
                        nc.vector.tensor_copy(zsl, psz_t[:])
                    else:
                        nc.vector.tensor_add(zsl, zsl, psz_t[:])

                def emit_out_sq(sq, qi):
                    ssl = bass.ts(sq, SC)
                    for ob in range(NDB):
                        pso = psz.tile([P, SC], f32, tag=f"z{(qi + ob) % 2}")
                        for eh in range(NDB):
                            nc.tensor.matmul(
                                pso[:],
                                wo_t[:, eh, bass.ts(ob, P)],
                                zt[:, eh, ssl],
                                start=(eh == 0),
                                stop=(eh == NDB - 1),
                            )
                        osb = opool.tile(
                            [P, SC], f32, tag=f"osb{ob}", name=f"osb{ob}"
                        )
                        nc.vector.tensor_scalar_add(osb[:], pso[:], bo_t[:, ob, :])
                        dma_eng = nc.sync if (sq + ob) % 2 == 0 else nc.gpsimd
                        dma_eng.dma_start(
                            out=outT[b, bass.ts(ob, P), ssl], in_=osb[:]
                        )

                def new_tiles():
                    return (
                        ppool.tile([P, G, S], bf16, tag="pt", name="pt"),
                        ppool.tile([P, G, D], bf16, tag="vp", name="vp"),
                        spool.tile([P, G, NSH], f32, tag="dnp", name="dnp"),
                    )

                prev = None
                for g in range(NSUP):
                    cur = new_tiles()
                    emit_scores_j(g, cur, 0)
                    for j in range(1, G):
                        emit_scores_j(g, cur, j)
                        if prev is not None:
                            emit_zt_q(g - 1, prev, 2 * (j - 1))
                            emit_zt_q(g - 1, prev, 2 * (j - 1) + 1)
                    emit_norm(g, cur)
                    if prev is not None:
                        emit_zt_q(g - 1, prev, 6)
                        emit_zt_q(g - 1, prev, 7)
                    prev = cur
                # next batch's x-load + projections fill the exp/norm tail of
                # the last superblock
                nxt = next_cb() if next_cb is not None else None
                # final superblock: sq-major quarters, out-projection chunks
                # lagging one sq behind
                for sq in range(NSC):
                    for eh in range(NDB):
                        emit_zt_q(NSUP - 1, prev, eh * NSC + sq)
                    if sq > 0:
                        emit_out_sq(sq - 1, 2 * sq)
                emit_out_sq(NSC - 1, 0)
                return nxt

            # ---- software pipeline over batches ----
            xt = load_x(0)
            proj = proj_qkv(0, xt)
            for b in range(B):
                if b + 1 < B:

                    def next_cb(bb=b + 1):
                        return proj_qkv(bb, load_x(bb))

                else:
                    next_cb = None
                proj = run_batch(b, *proj, next_cb)

    nc.compile()
    return nc


_NC = None


def _get_nc():
    global _NC
    if _NC is None:
        _NC = _build()
    return _NC


def _make_in_maps(x, Wq, bq, Wk, bk, Wv, bv, Wo, bo):
    x = np.asarray(x, np.float32)
    scale = np.float32(1.0 / np.sqrt(D))
    xT = np.ascontiguousarray(x.transpose(0, 2, 1).astype(ml_dtypes.bfloat16))
    in_maps = []
    for h in range(H):
        bvh = np.asarray(bv, np.float32)[h]
        m = {
            "xT": xT,
            "wqT": np.ascontiguousarray((np.asarray(Wq, np.float32)[h].T * scale).astype(ml_dtypes.bfloat16)),
            "wkT": np.ascontiguousarray(np.asarray(Wk, np.float32)[h].T.astype(ml_dtypes.bfloat16)),
            "wvT": np.ascontiguousarray(np.asarray(Wv, np.float32)[h].T.astype(ml_dtypes.bfloat16)),
            "woT": np.ascontiguousarray(
                np.asarray(Wo, np.float32)[:, h * D : (h + 1) * D].T.astype(
                    ml_dtypes.bfloat16
                )
            ),
            "bkc": np.ascontiguousarray(np.asarray(bk, np.float32)[h].reshape(D, 1)),
            "bvb": np.ascontiguousarray(
                np.broadcast_to(np.tile(bvh, VG), (P, VG * D)), dtype=np.float32
            ),
            "boc": np.ascontiguousarray(
                (
                    np.asarray(bo, np.float32) if h == 0 else np.zeros(D, np.float32)
                ).reshape(D, 1)
            ),
        }
        in_maps.append(m)
    return in_maps


def kernel(x, Wq, bq, Wk, bk, Wv, bv, Wo, bo, _trace=False, _trace_kwargs=None):
    in_maps = _make_in_maps(x, Wq, bq, Wk, bk, Wv, bv, Wo, bo)
    nc = _get_nc()
    kw = {}
    if _trace:
        kw = dict(trace=True, **(_trace_kwargs or {}))
    br = run_bass_kernel_spmd(nc, in_maps, core_ids=list(range(N_CORES)), **kw)
    acc = np.zeros((B, D, S), np.float32)
    for r in br.results:
        acc += r["outT"]
    out = np.ascontiguousarray(acc.transpose(0, 2, 1))
    if _trace:
        kernel.last_results = br
    return out
